# revision 1
# baseline (speedup 1.0000x reference)
"""Trainium2 Bass kernel for nn_ExactDivergenceModel (retrieval_knn).

Math (per batch b):
  XX[i,j] = ||X[i]-X[j]||, YX[i,j] = ||X[i]-Y[j]||
  out[b]  = (1/N) sum_i ( log min_{j!=i} XX[i,j] - log min_j YX[i,j] )
which only needs per-row minima of the squared-distance matrices:
  d2_XX[i,j] = x2[j] - 2<X_i,X_j>  (+ x2[i] added on host)
  d2_YX[i,j] = y2[j] - 2<X_i,Y_j>  (+ x2[i] added on host)

Device strategy (1 batch per NeuronCore, 8 cores):
  - Augmented matmul, K = D+2 = 66: lhsT = [-2*X^T; 1; 1], rhs = [Y^T; y2_hi; y2_lo]
    so PSUM directly holds y2[j] - 2<X_i, Y_j>. fp32r matmuls (1 cyc/row).
  - Diagonal of XX masked by accumulating BIG*I via a second matmul
    (lhsT = rhs = sqrt(BIG)*I_128, start=False) - PE-only, no vector cost.
  - Row minima via one VectorE tensor_reduce(min) per full PSUM row [128, 4096].
  - Host adds x2[i], applies eps clamp + log + mean in float64.
  - Default path is a raw-bacc build (_build_raw): semaphores ride on the
    compute instructions, eliminating the Tile scheduler's 128 standalone
    per-iteration EventSemaphore instructions (this backend's cost is
    dominated by per-instruction overhead). Tile build kept as fallback.
"""
import sys, time
sys.path.insert(0, '/opt/trn_rl_repo')

import numpy as np
import ml_dtypes

import concourse.bass as bass
import concourse.tile as tile
from concourse import bacc, mybir
from concourse.bass_utils import run_bass_kernel_spmd

B, N, D = 8, 4096, 64
P = 128                 # partitions / i-block size
NB = N // P             # 32 i-blocks
K = D + 2               # contraction with the two norm rows
HALF = 2048             # psum half-row width
EPS = 1e-12
SQRT_BIG = 32768.0      # BIG = 2^30 on the XX diagonal

_cache = {}

MM_DTYPE = "float32r"   # "float32r" | "float16" | "bfloat16" | "bf16x2"
MM_W = 512              # matmul free-dim width (chunk)
RED_W = 4096            # reduce width (psum tile width)


def _build(repeat=1, mmdt_name=None, mm_w=None, red_w=None, skip_reduce=False,
           skip_mm=False):
    mmdt_name = mmdt_name or MM_DTYPE
    mm_w = mm_w or MM_W
    red_w = red_w or RED_W
    assert red_w % mm_w == 0 and N % red_w == 0
    n_tiles = N // red_w            # psum tiles per (block, matrix)
    n_ch = red_w // mm_w            # matmuls per psum tile
    psum_bufs = 1 if red_w == 4096 else 2
    nc = bacc.Bacc(None, target_bir_lowering=False)
    f32 = mybir.dt.float32
    mmdt = None if mmdt_name == "bf16x2" else getattr(mybir.dt, mmdt_name)

    bf16x2 = mmdt_name == "bf16x2"
    if bf16x2:
        K1, K2 = D + 2, 2 * D
        L_d = nc.dram_tensor("L", [K1, N], f32, kind="ExternalInput")     # [-2Xhi^T; 1]
        RX_d = nc.dram_tensor("RX", [K1, N], f32, kind="ExternalInput")   # [Xhi^T; x2h]
        RY_d = nc.dram_tensor("RY", [K1, N], f32, kind="ExternalInput")
        L2_d = nc.dram_tensor("L2", [K2, N], f32, kind="ExternalInput")   # [-2Xhi^T; -2Xlo^T; 1]
        RX2_d = nc.dram_tensor("RX2", [K2, N], f32, kind="ExternalInput") # [Xlo^T; Xhi^T; x2lo]
        RY2_d = nc.dram_tensor("RY2", [K2, N], f32, kind="ExternalInput")
        mmdt = mybir.dt.bfloat16
    else:
        RX_d = nc.dram_tensor("RX", [K, N], f32, kind="ExternalInput")
        RY_d = nc.dram_tensor("RY", [K, N], f32, kind="ExternalInput")
    EYE_d = nc.dram_tensor("EYE", [P, P], f32, kind="ExternalInput")
    MX_d = nc.dram_tensor("MX", [P, n_tiles * NB], f32, kind="ExternalOutput")
    MY_d = nc.dram_tensor("MY", [P, n_tiles * NB], f32, kind="ExternalOutput")

    with tile.TileContext(nc) as tc:
        with tc.tile_pool(name="const", bufs=1) as const, \
             tc.tile_pool(name="psum", bufs=psum_bufs, space="PSUM") as psum, \
             tc.tile_pool(name="outs", bufs=1) as outs:
            KA = (D + 2) if bf16x2 else K
            Lf = const.tile([KA, N], f32)
            RXf = const.tile([KA, N], f32)
            RYf = const.tile([KA, N], f32)
            EYEf = const.tile([P, P], f32)
            if bf16x2:
                nc.sync.dma_start(out=Lf, in_=L_d[:])
            else:
                # L = [-2*X^T; 1; 1] derived from RX = [X^T; x2h; x2l]
                nc.vector.memset(Lf[D:D + 2, :], 1.0)
            nc.sync.dma_start(out=RXf, in_=RX_d[:])
            nc.sync.dma_start(out=RYf, in_=RY_d[:])
            nc.sync.dma_start(out=EYEf, in_=EYE_d[:])
            if not bf16x2:
                nc.vector.tensor_scalar_mul(Lf[0:D, :], RXf[0:D, :], -2.0)

            Lr = const.tile([KA, N], mmdt)
            RXr = const.tile([KA, N], mmdt)
            RYr = const.tile([KA, N], mmdt)
            EYEr = const.tile([P, P], mmdt)
            nc.vector.tensor_copy(Lr, Lf)
            nc.vector.tensor_copy(RXr, RXf)
            nc.vector.tensor_copy(RYr, RYf)
            nc.vector.tensor_copy(EYEr, EYEf)
            if bf16x2:
                L2f = const.tile([K2, N], f32)
                RX2f = const.tile([K2, N], f32)
                RY2f = const.tile([K2, N], f32)
                nc.sync.dma_start(out=L2f, in_=L2_d[:])
                nc.sync.dma_start(out=RX2f, in_=RX2_d[:])
                nc.sync.dma_start(out=RY2f, in_=RY2_d[:])
                L2r = const.tile([K2, N], mmdt)
                RX2r = const.tile([K2, N], mmdt)
                RY2r = const.tile([K2, N], mmdt)
                nc.vector.tensor_copy(L2r, L2f)
                nc.vector.tensor_copy(RX2r, RX2f)
                nc.vector.tensor_copy(RY2r, RY2f)

            mins_x = outs.tile([P, n_tiles * NB], f32)
            mins_y = outs.tile([P, n_tiles * NB], f32)

            for _ in range(repeat):
                for bi in range(NB):
                    lhs = Lr[:, bi * P:(bi + 1) * P]
                    if bf16x2:
                        mats = ((RXr, RX2r, mins_x, True), (RYr, RY2r, mins_y, False))
                    else:
                        mats = ((RXr, None, mins_x, True), (RYr, None, mins_y, False))
                    for R, R2, mins, is_xx in mats:
                        for h in range(n_tiles):
                            p = psum.tile([P, red_w], f32, tag="p")
                            for c in range(n_ch):
                                col0 = h * red_w + c * mm_w
                                diag_here = is_xx and col0 <= bi * P < col0 + mm_w
                                sl = p[:, c * mm_w:(c + 1) * mm_w]
                                last = not (diag_here or bf16x2)
                                if skip_mm and not (bi == 0 and c == 0):
                                    continue
                                nc.tensor.matmul(
                                    sl, lhs, R[:, col0:col0 + mm_w],
                                    start=True, stop=last)
                                if bf16x2:
                                    nc.tensor.matmul(
                                        sl, L2r[:, bi * P:(bi + 1) * P],
                                        R2[:, col0:col0 + mm_w],
                                        start=False, stop=not diag_here,
                                        skip_group_check=True)
                                if diag_here:
                                    off = c * mm_w + (bi * P - col0)
                                    nc.tensor.matmul(
                                        p[:, off:off + P], EYEr[:], EYEr[:],
                                        start=False, stop=True,
                                        skip_group_check=True)
                            if skip_reduce:
                                nc.vector.tensor_reduce(
                                    out=mins[:, n_tiles * bi + h:n_tiles * bi + h + 1],
                                    in_=p[:, 0:2], axis=mybir.AxisListType.X,
                                    op=mybir.AluOpType.min)
                            else:
                                nc.vector.tensor_reduce(
                                    out=mins[:, n_tiles * bi + h:n_tiles * bi + h + 1],
                                    in_=p[:], axis=mybir.AxisListType.X,
                                    op=mybir.AluOpType.min)

            nc.sync.dma_start(out=MX_d[:], in_=mins_x)
            nc.sync.dma_start(out=MY_d[:], in_=mins_y)

    nc.finalize()
    return nc




def _build_raw(repeat=1, use_fp32=False):
    """Raw-bacc variant: no Tile framework, sems ride on compute instructions.

    Cuts the 128 standalone per-iteration EventSemaphore instructions the Tile
    scheduler emits (2 per PSUM tile). Handshake per psum tile t (0..64R-1):
      PE: first matmul of tile t waits dve_sem >= t (psum free), last matmul
          then_inc(pe_sem).  DVE: reduce t waits pe_sem >= t+1, then_inc(dve_sem).
    """
    from contextlib import ExitStack
    f32 = mybir.dt.float32
    f32r = mybir.dt.float32r
    nc = bacc.Bacc(None, target_bir_lowering=False)
    RX_d = nc.dram_tensor("RX", [K, N], f32, kind="ExternalInput")
    RY_d = nc.dram_tensor("RY", [K, N], f32, kind="ExternalInput")
    EYE_d = nc.dram_tensor("EYE", [P, P], f32, kind="ExternalInput")
    MX_d = nc.dram_tensor("MX", [P, NB], f32, kind="ExternalOutput")
    MY_d = nc.dram_tensor("MY", [P, NB], f32, kind="ExternalOutput")

    n_tiles_total = 2 * NB * repeat

    with ExitStack() as ctx:
        RXf = ctx.enter_context(nc.sbuf_tensor([K, N], f32))
        RYf = ctx.enter_context(nc.sbuf_tensor([K, N], f32))
        Lf = ctx.enter_context(nc.sbuf_tensor([K, N], f32))
        EYEf = ctx.enter_context(nc.sbuf_tensor([P, P], f32))
        Lr = ctx.enter_context(nc.sbuf_tensor([K, N], f32r))
        RXr = ctx.enter_context(nc.sbuf_tensor([K, N], f32r))
        RYr = ctx.enter_context(nc.sbuf_tensor([K, N], f32r))
        EYEr = ctx.enter_context(nc.sbuf_tensor([P, P], mybir.dt.bfloat16))
        mins_x = ctx.enter_context(nc.sbuf_tensor([P, NB], f32))
        mins_y = ctx.enter_context(nc.sbuf_tensor([P, NB], f32))
        psum = ctx.enter_context(nc.psum_tensor([P, N], f32))
        dma_sem = ctx.enter_context(nc.semaphore())
        conv_sem = ctx.enter_context(nc.semaphore())
        pe_sem = ctx.enter_context(nc.semaphore())
        dve_sem = ctx.enter_context(nc.semaphore())
        block = ctx.enter_context(nc.Block())

        @block.sync
        def _(sync):
            sync.dma_start(out=RXf[:], in_=RX_d[:]).then_inc(dma_sem, 16)
            sync.dma_start(out=RYf[:], in_=RY_d[:]).then_inc(dma_sem, 16)
            sync.dma_start(out=EYEf[:], in_=EYE_d[:]).then_inc(dma_sem, 16)
            sync.wait_ge(dve_sem, n_tiles_total)
            sync.dma_start(out=MX_d[:], in_=mins_x[:]).then_inc(dma_sem, 16)
            sync.dma_start(out=MY_d[:], in_=mins_y[:]).then_inc(dma_sem, 16)

        if use_fp32:
            Lr, RXr, RYr, EYEr = Lf, RXf, RYf, EYEf

        @block.vector
        def _(vector):
            vector.wait_ge(dma_sem, 48)
            vector.memset(Lf[D:D + 2, :], 1.0)
            mul = nc.vector.tensor_scalar_mul(Lf[0:D, :], RXf[0:D, :], -2.0)
            if use_fp32:
                mul.then_inc(conv_sem, 1)
            else:
                nc.vector.tensor_copy(Lr[:], Lf[:])
                nc.vector.tensor_copy(RXr[:], RXf[:])
                nc.vector.tensor_copy(RYr[:], RYf[:])
                nc.vector.tensor_copy(EYEr[:], EYEf[:]).then_inc(conv_sem, 1)
            t = 0
            for _ in range(repeat):
                for bi in range(NB):
                    for mins in (mins_x, mins_y):
                        vector.wait_ge(pe_sem, t + 1)
                        nc.vector.tensor_reduce(
                            out=mins[:, bi:bi + 1], in_=psum[:],
                            axis=mybir.AxisListType.X,
                            op=mybir.AluOpType.min).then_inc(dve_sem, 1)
                        t += 1

        @block.tensor
        def _(tensor):
            tensor.wait_ge(conv_sem, 1)
            t = 0
            for _ in range(repeat):
                for bi in range(NB):
                    lhs = Lr[:, bi * P:(bi + 1) * P]
                    for R, is_xx in ((RXr, True), (RYr, False)):
                        if t > 0:
                            tensor.wait_ge(dve_sem, t)
                        for c in range(8):
                            col0 = c * 512
                            diag_here = is_xx and col0 <= bi * P < col0 + 512
                            mm = nc.tensor.matmul(
                                psum[:, col0:col0 + 512],
                                lhs, R[:, col0:col0 + 512],
                                start=True, stop=not diag_here)
                            if diag_here:
                                off = bi * P
                                mm = nc.tensor.matmul(
                                    psum[:, off:off + P], EYEr[:], EYEr[:],
                                    start=False, stop=True,
                                    skip_group_check=True)
                            if c == 7:
                                mm.then_inc(pe_sem, 1)
                        t += 1

    nc.finalize()
    return nc


def _get_nc(repeat=1, mmdt_name=None, mm_w=None, red_w=None):
    key = (repeat, mmdt_name or MM_DTYPE, mm_w or MM_W, red_w or RED_W)
    if key not in _cache:
        _cache[key] = _build(repeat, mmdt_name, mm_w, red_w)
    return _cache[key]


def _get_raw_nc(repeat=1):
    key = ("raw", repeat)
    if key not in _cache:
        _cache[key] = _build_raw(repeat)
    return _cache[key]


def _hi_round(v):
    # hi part must be exactly representable in the matmul dtype
    if MM_DTYPE == "float16":
        return v.astype(np.float32).astype(np.float16).astype(np.float64)
    return v.astype(np.float32).astype(ml_dtypes.bfloat16).astype(np.float64)


def _bf16(v):
    return v.astype(np.float32).astype(ml_dtypes.bfloat16).astype(np.float64)


def _prep_maps(X, Y):
    X = np.asarray(X, dtype=np.float32)
    Y = np.asarray(Y, dtype=np.float32)
    eye = (np.eye(P) * SQRT_BIG).astype(np.float32)
    in_maps = []
    x2_all = []
    for b in range(B):
        Xb = X[b].astype(np.float64)
        Yb = Y[b].astype(np.float64)
        x2 = (Xb * Xb).sum(1)
        y2 = (Yb * Yb).sum(1)
        ones = np.ones((1, N), dtype=np.float64)
        if MM_DTYPE == "bf16x2":
            Xh = _bf16(Xb); Xl = Xb - Xh
            Yh = _bf16(Yb); Yl = Yb - Yh
            x2h = _bf16(x2); y2h = _bf16(y2)
            L = np.concatenate([-2.0 * Xh.T, ones, ones], 0).astype(np.float32)
            RX = np.concatenate([Xh.T, x2h[None], (x2 - x2h)[None]], 0).astype(np.float32)
            RY = np.concatenate([Yh.T, y2h[None], (y2 - y2h)[None]], 0).astype(np.float32)
            L2 = np.concatenate([-2.0 * Xh.T, -2.0 * Xl.T], 0).astype(np.float32)
            RX2 = np.concatenate([Xl.T, Xh.T], 0).astype(np.float32)
            RY2 = np.concatenate([Yl.T, Yh.T], 0).astype(np.float32)
            in_maps.append({"L": L, "RX": RX, "RY": RY,
                            "L2": L2, "RX2": RX2, "RY2": RY2, "EYE": eye})
        else:
            x2h = _hi_round(x2)
            y2h = _hi_round(y2)
            RX = np.concatenate([Xb.T, x2h[None], (x2 - x2h)[None]], 0).astype(np.float32)
            RY = np.concatenate([Yb.T, y2h[None], (y2 - y2h)[None]], 0).astype(np.float32)
            in_maps.append({"RX": RX, "RY": RY, "EYE": eye})
        x2_all.append(x2)
    return in_maps, x2_all


def _postprocess(results, x2_all):
    out = np.zeros(B, dtype=np.float64)
    for b in range(B):
        mx = results[b]["MX"].astype(np.float64)  # [P, n_tiles*NB]
        my = results[b]["MY"].astype(np.float64)
        nt = mx.shape[1] // NB
        # [p, bi, h] -> min over tiles -> [p, bi] -> row i = bi*P + p
        d2x = mx.reshape(P, NB, nt).min(2).T.reshape(-1)
        d2y = my.reshape(P, NB, nt).min(2).T.reshape(-1)
        d2x = d2x + x2_all[b]
        d2y = d2y + x2_all[b]
        d2x = np.maximum(d2x, EPS)
        d2y = np.maximum(d2y, EPS)
        out[b] = 0.5 * np.mean(np.log(d2x) - np.log(d2y))
    return out.astype(np.float32)


def _run_with_retry(nc, in_maps):
    for attempt in range(2):
        try:
            return run_bass_kernel_spmd(nc, in_maps, core_ids=list(range(B))).results
        except Exception:
            time.sleep(3)
    # last resort: one batch at a time, skipping wedged cores
    results = [None] * B
    for b in range(B):
        for c in range(8):
            core = (b + c) % 8
            try:
                results[b] = run_bass_kernel_spmd(
                    nc, [in_maps[b]], core_ids=[core]).results[0]
                break
            except Exception:
                continue
        if results[b] is None:
            raise RuntimeError("all cores failed")
    return results


def kernel(X, Y):
    in_maps, x2_all = _prep_maps(X, Y)
    try:
        results = _run_with_retry(_get_raw_nc(1), in_maps)
    except Exception:
        # fall back to the Tile-framework build
        results = _run_with_retry(_get_nc(repeat=1), in_maps)
    return _postprocess(results, x2_all)


# Pre-build the default program at import time so the first kernel() call
# doesn't pay Bass graph construction; guarded so import can never fail.
try:
    _get_raw_nc(1)
except Exception:
    pass


if __name__ == "__main__":
    rng = np.random.default_rng(0)
    X = rng.standard_normal((B, N, D)).astype(np.float32)
    Y = rng.standard_normal((B, N, D)).astype(np.float32)
    print(kernel(X, Y))



# revision 2
# speedup vs baseline: 136.3608x; 136.3608x over previous
"""Trainium2 Bass kernel for nn_ExactDivergenceModel (retrieval_knn).

Math (per batch b):
  XX[i,j] = ||X[i]-X[j]||, YX[i,j] = ||X[i]-Y[j]||
  out[b]  = (1/N) sum_i ( log min_{j!=i} XX[i,j] - log min_j YX[i,j] )
which only needs per-row minima of the squared-distance matrices:
  d2_XX[i,j] = x2[j] - 2<X_i,X_j>  (+ x2[i] added on host)
  d2_YX[i,j] = y2[j] - 2<X_i,Y_j>  (+ x2[i] added on host)

Device strategy (1 batch per NeuronCore, 8 cores):
  - Augmented matmul, K = D+2 = 66: lhsT = [-2*X^T; 1; 1], rhs = [R^T; r2_hi;
    r2_lo] so PSUM directly holds r2[j] - 2<X_i, R_j>. fp32r matmuls.
  - Diagonal of XX masked by accumulating BIG*I via an extra matmul
    (lhsT = rhs = sqrt(BIG)*I_128, start=False) - PE-only, no vector cost.
  - PSUM is split into two [128, 2048] slots used round-robin: the PE fills
    slot t%2 while the DVE min-reduces slot (t-1)%2 -> PE and DVE overlap
    instead of strictly alternating (the single-slot [128,4096] layout
    serializes them and measures ~2.2x slower).
  - Row minima via VectorE tensor_reduce(min); host combines the two
    half-row minima, adds x2[i], applies eps clamp + log + mean in float64.
  - Raw-bacc build: semaphores ride on the compute instructions
    (then_inc) with standalone waits only; no Tile-scheduler overhead.

Execution: a jitted shard_map(bass_exec) callable is cached at module level
so repeat kernel() calls skip retracing/relowering (the lowering embeds the
NEFF and costs ~0.5 s per call otherwise). Fallback path goes through
run_bass_kernel_spmd, then per-core retry.
"""
import sys, time
sys.path.insert(0, '/opt/trn_rl_repo')

import numpy as np

import concourse.bass as bass
from concourse import bacc, mybir
from concourse.bass_utils import run_bass_kernel_spmd

B, N, D = 8, 4096, 64
P = 128                 # partitions / i-block size
NB = N // P             # 32 i-blocks
K = D + 2               # contraction with the two norm rows
HALF = 2048             # psum slot width (two slots)
MMW = 512               # matmul free-dim width (one PSUM bank)
EPS = 1e-12
SQRT_BIG = 32768.0      # BIG = 2^30 on the XX diagonal
f32 = mybir.dt.float32

_cache = {}


def _build(repeat=1):
    """Raw-bacc program. Per (block bi, matrix m in {XX, YX}) the [P, N]
    distance-row tile is computed as two [P, HALF] psum pieces; piece t goes
    to psum slot t%2, is min-reduced by DVE into mins[:, 2*bi+h], and the PE
    may refill a slot only after the reduce of the piece two steps back
    (wait dve_sem >= t-1), overlapping PE and DVE."""
    n_half = N // HALF              # 2
    n_ch = HALF // MMW              # 4
    mmdt = mybir.dt.float32r

    nc = bacc.Bacc(None, target_bir_lowering=False)
    RX_d = nc.dram_tensor("RX", [K, N], f32, kind="ExternalInput")
    RY_d = nc.dram_tensor("RY", [K, N], f32, kind="ExternalInput")
    EYE_d = nc.dram_tensor("EYE", [P, P], f32, kind="ExternalInput")
    MX_d = nc.dram_tensor("MX", [P, NB * n_half], f32, kind="ExternalOutput")
    MY_d = nc.dram_tensor("MY", [P, NB * n_half], f32, kind="ExternalOutput")

    n_tiles_total = 2 * NB * n_half * repeat

    from contextlib import ExitStack
    with ExitStack() as ctx:
        RXf = ctx.enter_context(nc.sbuf_tensor([K, N], f32))
        RYf = ctx.enter_context(nc.sbuf_tensor([K, N], f32))
        Lf = ctx.enter_context(nc.sbuf_tensor([K, N], f32))
        EYEf = ctx.enter_context(nc.sbuf_tensor([P, P], f32))
        Lr = ctx.enter_context(nc.sbuf_tensor([K, N], mmdt))
        RXr = ctx.enter_context(nc.sbuf_tensor([K, N], mmdt))
        RYr = ctx.enter_context(nc.sbuf_tensor([K, N], mmdt))
        EYEr = ctx.enter_context(nc.sbuf_tensor([P, P], mybir.dt.bfloat16))
        mins_x = ctx.enter_context(nc.sbuf_tensor([P, NB * n_half], f32))
        mins_y = ctx.enter_context(nc.sbuf_tensor([P, NB * n_half], f32))
        psum = ctx.enter_context(nc.psum_tensor([P, N], f32))
        dma_sem = ctx.enter_context(nc.semaphore())
        conv_sem = ctx.enter_context(nc.semaphore())
        pe_sem = ctx.enter_context(nc.semaphore())
        dve_sem = ctx.enter_context(nc.semaphore())
        block = ctx.enter_context(nc.Block())

        @block.sync
        def _(sync):
            sync.dma_start(out=RXf[:], in_=RX_d[:]).then_inc(dma_sem, 16)
            sync.dma_start(out=RYf[:], in_=RY_d[:]).then_inc(dma_sem, 16)
            sync.dma_start(out=EYEf[:], in_=EYE_d[:]).then_inc(dma_sem, 16)
            sync.wait_ge(dve_sem, n_tiles_total)
            sync.dma_start(out=MX_d[:], in_=mins_x[:]).then_inc(dma_sem, 16)
            sync.dma_start(out=MY_d[:], in_=mins_y[:]).then_inc(dma_sem, 16)

        @block.vector
        def _(vector):
            vector.wait_ge(dma_sem, 48)
            nc.vector.memset(Lf[D:D + 2, :], 1.0)
            nc.vector.tensor_scalar_mul(Lf[0:D, :], RXf[0:D, :], -2.0)
            nc.vector.tensor_copy(Lr[:], Lf[:])
            nc.vector.tensor_copy(RXr[:], RXf[:])
            nc.vector.tensor_copy(RYr[:], RYf[:])
            nc.vector.tensor_copy(EYEr[:], EYEf[:]).then_inc(conv_sem, 1)
            t = 0
            for _r in range(repeat):
                for bi in range(NB):
                    for mins in (mins_x, mins_y):
                        for h in range(n_half):
                            slot = t % 2
                            vector.wait_ge(pe_sem, t + 1)
                            col = bi * n_half + h
                            nc.vector.tensor_reduce(
                                out=mins[:, col:col + 1],
                                in_=psum[:, slot * HALF:(slot + 1) * HALF],
                                axis=mybir.AxisListType.X,
                                op=mybir.AluOpType.min).then_inc(dve_sem, 1)
                            t += 1

        @block.tensor
        def _(tensor):
            tensor.wait_ge(conv_sem, 1)
            t = 0
            for _r in range(repeat):
                for bi in range(NB):
                    lhs = Lr[:, bi * P:(bi + 1) * P]
                    for R, is_xx in ((RXr, True), (RYr, False)):
                        for h in range(n_half):
                            slot = t % 2
                            if t >= 2:
                                tensor.wait_ge(dve_sem, t - 1)
                            mm = None
                            for c in range(n_ch):
                                col0 = h * HALF + c * MMW
                                diag_here = (is_xx
                                             and col0 <= bi * P < col0 + MMW)
                                mm = nc.tensor.matmul(
                                    psum[:, slot * HALF + c * MMW:
                                         slot * HALF + (c + 1) * MMW],
                                    lhs, R[:, col0:col0 + MMW],
                                    start=True, stop=not diag_here)
                                if diag_here:
                                    off = slot * HALF + (bi * P - h * HALF)
                                    mm = nc.tensor.matmul(
                                        psum[:, off:off + P], EYEr[:], EYEr[:],
                                        start=False, stop=True,
                                        skip_group_check=True)
                            mm.then_inc(pe_sem, 1)
                            t += 1

    nc.finalize()
    return nc


def _get_nc(repeat=1):
    key = ("raw2", repeat)
    if key not in _cache:
        _cache[key] = _build(repeat)
    return _cache[key]


def _prep_maps(X, Y):
    X = np.asarray(X, dtype=np.float32)
    Y = np.asarray(Y, dtype=np.float32)
    eye = (np.eye(P) * SQRT_BIG).astype(np.float32)
    in_maps, x2_all = [], []
    for b in range(B):
        Xb = X[b].astype(np.float64)
        Yb = Y[b].astype(np.float64)
        x2 = (Xb * Xb).sum(1)
        y2 = (Yb * Yb).sum(1)
        # hi part must be exactly representable in bf16 (f32r rounds via bf16
        # passes); keep the residual in a second augmented row.
        import ml_dtypes
        x2h = x2.astype(np.float32).astype(ml_dtypes.bfloat16).astype(np.float64)
        y2h = y2.astype(np.float32).astype(ml_dtypes.bfloat16).astype(np.float64)
        RX = np.concatenate([Xb.T, x2h[None], (x2 - x2h)[None]], 0).astype(np.float32)
        RY = np.concatenate([Yb.T, y2h[None], (y2 - y2h)[None]], 0).astype(np.float32)
        in_maps.append({"RX": RX, "RY": RY, "EYE": eye})
        x2_all.append(x2)
    return in_maps, x2_all


def _postprocess(results, x2_all):
    n_half = N // HALF
    out = np.zeros(B, dtype=np.float64)
    for b in range(B):
        mx = results[b]["MX"].astype(np.float64)  # [P, NB*n_half]
        my = results[b]["MY"].astype(np.float64)
        d2x = mx.reshape(P, NB, n_half).min(2).T.reshape(-1) + x2_all[b]
        d2y = my.reshape(P, NB, n_half).min(2).T.reshape(-1) + x2_all[b]
        d2x = np.maximum(d2x, EPS)
        d2y = np.maximum(d2y, EPS)
        out[b] = 0.5 * np.mean(np.log(d2x) - np.log(d2y))
    return out.astype(np.float32)


# ---------------------------------------------------------------------------
# Cached jitted runner: build the shard_map(bass_exec) callable once; only
# device_put of the (possibly new) inputs + execute happens per call.

def _make_runner(nc, n_cores=B):
    import jax
    from jax.sharding import Mesh, PartitionSpec, NamedSharding
    from jax.experimental.shard_map import shard_map
    from concourse.bass2jax import (
        _bass_exec_p, install_neuronx_cc_hook, partition_id_tensor)

    install_neuronx_cc_hook()
    partition_name = (nc.partition_id_tensor.name
                      if nc.partition_id_tensor else None)
    in_names, out_names, out_avals, zero_outs = [], [], [], []
    for alloc in nc.m.functions[0].allocations:
        if not isinstance(alloc, mybir.MemoryLocationSet):
            continue
        name = alloc.memorylocations[0].name
        if alloc.kind == "ExternalInput":
            if name != partition_name:
                in_names.append(name)
        elif alloc.kind == "ExternalOutput":
            out_names.append(name)
            shape = tuple(alloc.tensor_shape)
            dtype = mybir.dt.np(alloc.dtype)
            out_avals.append(jax.core.ShapedArray(shape, dtype))
            zero_outs.append(np.zeros(shape, dtype))
    n_params = len(in_names)
    in_names_all = list(in_names) + out_names
    if partition_name is not None:
        in_names_all.append(partition_name)

    def _body(*args):
        operands = list(args)
        if partition_name is not None:
            operands.append(partition_id_tensor())
        outs = _bass_exec_p.bind(
            *operands,
            out_avals=tuple(out_avals),
            in_names=tuple(in_names_all),
            out_names=tuple(out_names),
            lowering_input_output_aliases=(),
            sim_require_finite=True,
            sim_require_nnan=True,
            nc=nc,
        )
        return tuple(outs)

    devices = jax.devices()[:n_cores]
    mesh = Mesh(np.asarray(devices), ("core",))
    in_specs = (PartitionSpec("core"),) * (n_params + len(out_names))
    out_specs = (PartitionSpec("core"),) * len(out_names)
    fn = jax.jit(
        shard_map(_body, mesh=mesh, in_specs=in_specs, out_specs=out_specs,
                  check_rep=False),
        keep_unused=True,
    )
    sharding = NamedSharding(mesh, PartitionSpec("core"))
    dev_zeros = [
        jax.device_put(
            np.zeros((n_cores * z.shape[0], *z.shape[1:]), z.dtype), sharding)
        for z in zero_outs
    ]
    state = dict(fn=fn, in_names=in_names, out_names=out_names,
                 out_avals=out_avals, sharding=sharding, dev_zeros=dev_zeros,
                 n_cores=n_cores)
    return state


def _get_runner(repeat=1):
    key = ("runner", repeat)
    if key not in _cache:
        _cache[key] = _make_runner(_get_nc(repeat))
    return _cache[key]


def _run_cached(in_maps):
    import jax
    st = _get_runner(1)
    n_cores = st["n_cores"]
    concat_in = [
        np.concatenate([np.asarray(in_maps[c][name])
                        for c in range(n_cores)], axis=0)
        for name in st["in_names"]
    ]
    dev_in = [jax.device_put(a, st["sharding"]) for a in concat_in]
    out = st["fn"](*dev_in, *st["dev_zeros"])
    jax.block_until_ready(out)
    return [
        {name: np.asarray(out[i]).reshape(n_cores, *st["out_avals"][i].shape)[c]
         for i, name in enumerate(st["out_names"])}
        for c in range(n_cores)
    ]


def _run_with_retry(nc, in_maps):
    for attempt in range(2):
        try:
            return run_bass_kernel_spmd(nc, in_maps,
                                        core_ids=list(range(B))).results
        except Exception:
            time.sleep(3)
    # last resort: one batch at a time, skipping wedged cores
    results = [None] * B
    for b in range(B):
        for c in range(8):
            core = (b + c) % 8
            try:
                results[b] = run_bass_kernel_spmd(
                    nc, [in_maps[b]], core_ids=[core]).results[0]
                break
            except Exception:
                continue
        if results[b] is None:
            raise RuntimeError("all cores failed")
    return results


def kernel(X, Y):
    in_maps, x2_all = _prep_maps(X, Y)
    try:
        results = _run_cached(in_maps)
    except Exception:
        results = _run_with_retry(_get_nc(1), in_maps)
    return _postprocess(results, x2_all)


# Pre-build the program at import time so the first kernel() call doesn't pay
# Bass graph construction; guarded so import can never fail.
try:
    _get_nc(1)
except Exception:
    pass


if __name__ == "__main__":
    rng = np.random.default_rng(0)
    X = rng.standard_normal((B, N, D)).astype(np.float32)
    Y = rng.standard_normal((B, N, D)).astype(np.float32)
    print(kernel(X, Y))
